# revision 8
# baseline (speedup 1.0000x reference)
"""Trainium2 Bass kernel: Anscombe transform -> 3x3 Gaussian blur -> inverse
Anscombe, on a [1,4096,4096,3] fp32 image, sharded over H across 8 NeuronCores.

I/O is fp16 on the wire (host casts before upload / after download).

Per core (512 output rows): 4 blocks of 126 rows + one folded 8-row runt.

Main blocks:
  DMA in (3 column pieces of 4096, 8KB descriptors, alternating the two
  HWDGE queues; prefetched one block ahead)
  -> ACT: at = sqrt(4x + 1.5) in 4096-col pieces, software-pipelined into
     the PREVIOUS block's group loop so ACT never stalls the DVE (pad rows
     hold -0.375 so at = 0, matching the reference's zero padding)
  -> PE: full 3x3 conv as 3 accumulated fp16 matmuls per 512-col PSUM chunk
     (vertical taps via banded weight matrix over partitions, horizontal via
     +-3-column shifts of the interleaved-channel rhs).  Weights carry a
     global scale s = c^(-1/3) so PSUM holds ps = s*y, which makes the
     inverse-Anscombe cubic MONIC in r = 1/ps:
        a/y + b/y^2 + c/y^3 = r*(r - ka)*(r - kb)        (exactly)
     with ka,kb = (real roots of c t^2 + b t + a) / s.
  -> ACT: u = Square(0.5/s * ps) = 0.25*y^2
  -> DVE: r = reciprocal_approx_fast(ps)
  -> DVE: custom op  out = r*(r-ka)*(r-kb) + (u - 0.125)   (6 of 8 stages)
  -> DMA out (full-width rows, 24KB descriptors, gpsimd SWDGE).

Runt (8 rows x 12288 cols): folded to [120, 1030] / [96, 1024] tiles
(12 column-groups x 10 input rows on partitions) via rearranged DMA access
patterns, so its elementwise passes cost 1024 columns instead of 12288.
DVE/ACT/PE tile cost is free-size * cycle regardless of partition count, so
the unfolded runt wasted ~26us of DVE alone.
"""

import numpy as np
import ml_dtypes

import concourse.bass as bass
import concourse.bacc as bacc
import concourse.mybir as mybir
import concourse.tile as tile
from concourse import dve_ops
from concourse.bass_utils import run_bass_kernel_spmd
from concourse.dve_spec import C0, C1, C2, Spec, Src0, Src1, _has_src1
from concourse.dve_spec import lower as dve_lower
from concourse.dve_uop import DveOpSpec

F32 = mybir.dt.float32
FP16 = mybir.dt.float16

# ---------------------------------------------------------------- constants
H, W, CH = 4096, 4096, 3
WC = W * CH
N_CORES = 8
H_CORE = H // N_CORES          # output rows per core
BLOCK = 126                    # output rows per full block (128 input rows)
CHUNK = 512                    # matmul N (one PSUM bank)
GROUP = 2048                   # postprocess tile width (4 PSUM banks)
PIECE = 4096                   # input-DMA / sqrt column piece
PAD_VAL = -0.375               # sqrt affine maps this to exactly 0
SQRT_SCALE = 4.0               # at = sqrt(4x + 1.5) = 2*sqrt(x + 0.375)
SQRT_BIAS = 1.5

# runt folding: 8 output rows x 12288 cols -> 12 col-groups of 1024 on
# partitions p = 10*cg + row (input) / 8*cg + row (output)
R_M = H_CORE - 4 * BLOCK       # 8 runt output rows
R_KIN = R_M + 2                # 10 input rows
R_NG = 12                      # column groups
R_COLS = WC // R_NG            # 1024
R_PIN = R_NG * R_KIN           # 120 input partitions
R_POUT = R_NG * R_M            # 96 output partitions

# Gaussian kernel exactly as the reference builds it (fp32 throughout)
_coords = np.arange(-1, 2, dtype=np.float32)
_g = np.exp(-(_coords[:, None] ** 2 + _coords[None, :] ** 2)
            / (np.float32(2.0) * np.float32(1.3) ** 2)).astype(np.float32)
K2D = (_g / _g.sum()).astype(np.float32)       # [3,3], rows=dy, cols=dx

_s15 = np.sqrt(np.float64(1.5))
A_C = float(0.25 * _s15)            # coefficient of 1/y
B_C = float(-11.0 / 8.0)            # coefficient of 1/y^2
C_C = float(0.625 * _s15)           # coefficient of 1/y^3
S_PS = float(C_C ** (-1.0 / 3.0))   # PSUM = S_PS * y  (makes the cubic monic)
SQ_SCALE = float(0.5 / S_PS)        # Square(SQ_SCALE*ps) = 0.25*y^2
_disc = float(np.sqrt(B_C * B_C - 4.0 * C_C * A_C))
KA = float((-B_C + _disc) / (2.0 * C_C) / S_PS)
KB = float((-B_C - _disc) / (2.0 * C_C) / S_PS)


# ------------------------------------------------- custom DVE op (the tail)
def _register_tail_op():
    """out = Src0*(Src0-C0)*(Src0-C1) + (Src1 + C2); Src0=r, Src1=u.

    With r = 1/(s*y), u = 0.25*y^2, C0=ka, C1=kb, C2=-1/8 this is exactly
    0.25 y^2 - 0.125 + a/y + b/y^2 + c/y^3   (6 ALU stages of 8)."""
    name = "ANSCOMBE_TAIL_FACT_ANT"
    for op in dve_ops.OPS:
        if op.name == name:
            return op
    spec = Spec(
        body=Src0 * ((Src0 - C0) * (Src0 - C1)) + (Src1 + C2),
        reference=lambda in0, in1, c0, c1, c2: (
            in0.astype(np.float32)
            * ((in0 - np.float32(c0)) * (in0 - np.float32(c1)))
            + (in1.astype(np.float32) + np.float32(c2))
        ).astype(np.float32),
    )
    row = max(dve_ops._SUB_OPCODE_FOR_NAME.values()) + 1
    assert row < 0x20
    dve_ops._SUB_OPCODE_FOR_NAME[name] = row
    shas = {}
    for ver in ("v3", "v4"):
        ds = DveOpSpec(name=name, opcode=row, uops=dve_lower(spec, ver=ver),
                       rd1_en=_has_src1(spec))
        shas[ver] = ds.sha(ver)
    op = dve_ops.DveOp(name, spec, subdim=False, uops_sha=shas)
    dve_ops.OPS.append(op)
    dve_ops.CUSTOM_DVE_SPECS[name] = spec
    return op


def _weight_matrix():
    """[128, 3*BLOCK] fp16 band matrix: segment j (horizontal tap dx=j-1) has
    K2D[d, j]*S_PS on diagonal k-m = d (vertical tap dy=d-1)."""
    w = K2D.astype(np.float64) * S_PS   # [d, j]
    wm = np.zeros((128, 3 * BLOCK), dtype=np.float64)
    for j in range(3):
        for d in range(3):
            for m in range(BLOCK):
                wm[m + d, j * BLOCK + m] = w[d, j]
    return wm.astype(np.float16)


def _runt_weight_matrix():
    """[120, 3*96] fp16 block-banded matrix for the folded runt: out partition
    8*cg + m gets tap d from input partition 10*cg + m + d."""
    w = K2D.astype(np.float64) * S_PS
    wm = np.zeros((R_PIN, 3 * R_POUT), dtype=np.float64)
    for j in range(3):
        for cg in range(R_NG):
            for d in range(3):
                for m in range(R_M):
                    wm[R_KIN * cg + m + d, j * R_POUT + R_M * cg + m] = w[d, j]
    return wm.astype(np.float16)


# ------------------------------------------------------------- bass program
def build_nc(h_out=H_CORE, wc=WC):
    tail_op = _register_tail_op()
    h_in = h_out + 2
    nc = bacc.Bacc(None, target_bir_lowering=False)
    # const AP for the sqrt bias (activation converts float bias to an AP)
    _bias = nc.alloc_sbuf_tensor("const-sqrt-bias", [128, 1], F32)
    nc.gpsimd.memset(_bias.ap(), SQRT_BIAS)
    nc.const_aps.aps[(F32, SQRT_BIAS)] = _bias.ap()
    nc.all_engine_barrier()

    x = nc.declare_dram_parameter("x", [h_in, wc], FP16, isOutput=False)
    wmat = nc.declare_dram_parameter("wm", [128, 3 * BLOCK], FP16, isOutput=False)
    wmat2 = nc.declare_dram_parameter("wm2", [R_PIN, 3 * R_POUT], FP16,
                                      isOutput=False)
    out = nc.declare_dram_parameter("out", [h_out, wc], FP16, isOutput=True)

    n_blk = 4                       # full blocks; then the folded runt
    r0_runt = n_blk * BLOCK
    n_grp = wc // GROUP
    n_pc = wc // PIECE
    SQRT = mybir.ActivationFunctionType.Sqrt
    SQUARE = mybir.ActivationFunctionType.Square

    with tile.TileContext(nc) as tc:
        with (
            tc.tile_pool(name="consts", bufs=1) as cpool,
            tc.tile_pool(name="xpool", bufs=2) as xpool,
            tc.tile_pool(name="at", bufs=2) as atpool,
            tc.tile_pool(name="runt", bufs=1) as runtpool,
            tc.tile_pool(name="upool", bufs=2) as upool,
            tc.tile_pool(name="rpool", bufs=2) as rpool,
            tc.tile_pool(name="opool", bufs=2) as opool,
            tc.tile_pool(name="psum", bufs=2, space="PSUM") as pspool,
        ):
            wt = cpool.tile([128, 3 * BLOCK], FP16)
            nc.sync.dma_start(wt[:], wmat[:])
            wt2 = cpool.tile([R_PIN, 3 * R_POUT], FP16)
            nc.scalar.dma_start(wt2[:], wmat2[:])

            # block 0 uses small leading pieces so the first matmul group's
            # dependencies land ASAP; later blocks prefetch a whole block
            # ahead, so three even pieces suffice.
            PIECES0 = [0, GROUP + 6, 2 * GROUP + 6, 3 * GROUP + 6, wc]
            PIECES = [0, PIECE, 2 * PIECE, wc]

            def issue_main_input(bi):
                """DMA (column pieces, alternating queues) + border memsets."""
                r0 = bi * BLOCK
                k_in = BLOCK + 2
                xc = xpool.tile([128, wc], FP16, tag="xc")
                at = atpool.tile([128, wc + 6], FP16, tag="at")
                bounds = PIECES0 if bi == 0 else PIECES
                for k in range(len(bounds) - 1):
                    c0, c1 = bounds[k], bounds[k + 1]
                    eng = nc.sync if (bi + k) % 2 == 0 else nc.scalar
                    eng.dma_start(xc[:k_in, c0:c1], x[r0:r0 + k_in, c0:c1])
                nc.gpsimd.memset(at[:k_in, 0:3], 0.0)
                nc.gpsimd.memset(at[:k_in, wc + 3:wc + 6], 0.0)
                return xc, at

            def sqrt_piece(tiles, bi, k):
                xc, at = tiles
                bounds = PIECES0 if bi == 0 else PIECES
                c0, c1 = bounds[k], bounds[k + 1]
                nc.scalar.activation(at[:BLOCK + 2, 3 + c0:3 + c1],
                                     xc[:BLOCK + 2, c0:c1],
                                     SQRT, bias=SQRT_BIAS, scale=SQRT_SCALE)

            def issue_runt_input():
                """Folded runt input: [120, 1030] = 12 col-groups x 10 rows,
                3-col halos between groups; PAD_VAL in the outermost borders
                so sqrt maps them to exactly 0 (horizontal zero padding)."""
                x2 = runtpool.tile([R_PIN, R_COLS + 6], FP16, tag="x2")
                at2 = runtpool.tile([R_PIN, R_COLS + 6], FP16, tag="at2")
                nc.gpsimd.memset(x2[:, 0:3], PAD_VAL)
                nc.gpsimd.memset(x2[:, R_COLS + 3:R_COLS + 6], PAD_VAL)
                nc.scalar.dma_start(
                    x2[:, 3:3 + R_COLS],
                    x[r0_runt:r0_runt + R_KIN, :].rearrange(
                        "r (g c) -> g r c", g=R_NG))
                span = (R_NG - 1) * R_COLS
                nc.sync.dma_start(
                    x2[R_KIN:, 0:3],
                    x[r0_runt:r0_runt + R_KIN,
                      R_COLS - 3:R_COLS - 3 + span].rearrange(
                        "r (g c) -> g r c", g=R_NG - 1)[:, :, 0:3])
                nc.sync.dma_start(
                    x2[:(R_NG - 1) * R_KIN, R_COLS + 3:R_COLS + 6],
                    x[r0_runt:r0_runt + R_KIN,
                      R_COLS:R_COLS + span].rearrange(
                        "r (g c) -> g r c", g=R_NG - 1)[:, :, 0:3])
                return x2, at2

            def postprocess(ps, o_ap, m, width):
                u = upool.tile([BLOCK, GROUP], F32, tag="u")
                r = rpool.tile([BLOCK, GROUP], F32, tag="r")
                nc.scalar.activation(u[:m, :width], ps[:m, :width],
                                     SQUARE, scale=SQ_SCALE)
                nc.vector.reciprocal_approx_fast(out=r[:m, :width],
                                                 in_=ps[:m, :width])
                nc.vector._custom_dve(tail_op, out=o_ap,
                                      in0=r[:m, :width], in1=u[:m, :width],
                                      s0=KA, s1=KB, imm2=-0.125)

            def runt_compute():
                """Folded runt conv + postprocess + output (overlapped into
                block 3's group loop)."""
                _, at2 = tiles[n_blk]
                ps = pspool.tile([BLOCK, GROUP], F32, tag="ps")
                for j in range(3):
                    for c0 in range(0, R_COLS, CHUNK):
                        nc.tensor.matmul(
                            ps[:R_POUT, c0:c0 + CHUNK],
                            wt2[:R_PIN, j * R_POUT:(j + 1) * R_POUT],
                            at2[:R_PIN, c0 + 3 * j:c0 + 3 * j + CHUNK],
                            start=(j == 0), stop=(j == 2),
                        )
                o2 = opool.tile([BLOCK, wc], FP16, tag="o")
                postprocess(ps, o2[:R_POUT, :R_COLS], R_POUT, R_COLS)
                # SWDGE only: the 3-level DRAM dest pattern exceeds PDMA2D
                nc.gpsimd.dma_start(
                    out[r0_runt:r0_runt + R_M, :].rearrange(
                        "r (g c) -> g r c", g=R_NG),
                    o2[:R_POUT, :R_COLS])

            # ---- prime the pipeline
            tiles = [None] * (n_blk + 1)
            tiles[0] = issue_main_input(0)
            for k in range(len(PIECES0) - 1):
                sqrt_piece(tiles[0], 0, k)
            if n_blk > 1:
                tiles[1] = issue_main_input(1)

            # ---- main blocks
            for bi in range(n_blk):
                r0 = bi * BLOCK
                k_in = BLOCK + 2
                xc, at = tiles[bi]
                last = bi == n_blk - 1
                o = opool.tile([BLOCK, wc], FP16, tag="o")
                if bi == 2:
                    tiles[n_blk] = issue_runt_input()
                for g in range(n_grp):
                    g0 = g * GROUP
                    ps = pspool.tile([BLOCK, GROUP], F32, tag="ps")
                    # taps outer so consecutive matmuls share stationary weights
                    for j in range(3):
                        for c0 in range(0, GROUP, CHUNK):
                            nc.tensor.matmul(
                                ps[:BLOCK, c0:c0 + CHUNK],
                                wt[:k_in, j * BLOCK:(j + 1) * BLOCK],
                                at[:k_in, g0 + c0 + 3 * j:
                                   g0 + c0 + 3 * j + CHUNK],
                                start=(j == 0), stop=(j == 2),
                            )
                    postprocess(ps, o[:BLOCK, g0:g0 + GROUP], BLOCK, GROUP)
                    # software-pipelined ACT prep for the next block
                    if g % 2 == 0 and not last:
                        sqrt_piece(tiles[bi + 1], bi + 1, g // 2)
                    if last and g == 0:
                        x2, at2 = tiles[n_blk]
                        nc.scalar.activation(at2[:, :], x2[:, :], SQRT,
                                             bias=SQRT_BIAS, scale=SQRT_SCALE)
                    if last and g == 2:
                        runt_compute()
                        # first half of the last block's rows->DRAM early
                        nc.gpsimd.dma_start(out[r0:r0 + BLOCK, :wc // 2],
                                            o[:BLOCK, :wc // 2])
                if not last:
                    # full-width output rows via SWDGE (24KB descriptors)
                    nc.gpsimd.dma_start(out[r0:r0 + BLOCK, :], o[:BLOCK, :])
                else:
                    # final piece split across two queues to halve the drain
                    hb = BLOCK // 2
                    nc.gpsimd.dma_start(out[r0:r0 + hb, wc // 2:],
                                        o[:hb, wc // 2:])
                    nc.sync.dma_start(out[r0 + hb:r0 + BLOCK, wc // 2:],
                                      o[hb:BLOCK, wc // 2:])
                if bi + 2 < n_blk:
                    tiles[bi + 2] = issue_main_input(bi + 2)
    nc.compile()
    return nc


# ------------------------------------------------------------------- driver
_CACHE = {}


def _get_nc(h_out, wc):
    key = (h_out, wc)
    if key not in _CACHE:
        _CACHE[key] = build_nc(h_out, wc)
    return _CACHE[key]


def run_sharded(x2d, n_cores=N_CORES, trace=False, **kw):
    """x2d: [H, W*C] fp32 full image (2D). Returns ([H, W*C] fp32, results)."""
    h, wc = x2d.shape
    h_core = h // n_cores
    nc = _get_nc(h_core, wc)
    wm = _weight_matrix()
    wm2 = _runt_weight_matrix()
    in_maps = []
    for i in range(n_cores):
        lo, hi = i * h_core - 1, (i + 1) * h_core + 1
        src_lo, src_hi = max(lo, 0), min(hi, h)
        if lo < 0 or hi > h:
            slab = np.full((h_core + 2, wc), PAD_VAL, dtype=np.float16)
        else:
            slab = np.empty((h_core + 2, wc), dtype=np.float16)
        slab[src_lo - lo:src_hi - lo] = x2d[src_lo:src_hi]
        in_maps.append({"x": slab, "wm": wm, "wm2": wm2})
    res = run_bass_kernel_spmd(nc, in_maps, list(range(n_cores)), trace=trace, **kw)
    full = np.concatenate([res.results[i]["out"] for i in range(n_cores)],
                          axis=0).astype(np.float32)
    return full, res


def kernel(im: np.ndarray) -> np.ndarray:
    x2d = np.asarray(im, dtype=np.float32).reshape(H, WC)
    full, _ = run_sharded(x2d)
    return full.reshape(H, W, CH)
